# revision 92
# baseline (speedup 1.0000x reference)
"""Causal single-head attention on 8 TRN2 NeuronCores.

Problem: x[B=4,T=4096,D=2048] @ Wq/Wk/Wv[D,H=128] -> causal attention -> out[B,T,H].

Sharding (v6): 2 cores per batch, split by KEY parity with host-side softmax
combine. Core parity p owns the interleaved 128-row KEY blocks 2c+p
(c = 0..15): it computes K/V projections for its own key half only, Q for ALL
queries, scores+AV of every query against its own keys, and writes the
UNNORMALIZED partial numerator plus denominator [T, H+1]. The host sums the
two partials of each batch and divides. This duplicates the Q projection
(cheap, 1x) instead of K/V projection + transpose (2x+), cutting per-core PE
work ~17% vs the query-parity scheme (~203k PE cycles/core vs ~246k).

The host permutes each batch's rows to [own-parity 128-blocks | other
blocks], transposes and casts to bf16, so one 16MB xT stream feeds all three
projections (K/V over the first half only). The permuted causal structure is
core-independent; per-core causality lives in two 128x128 mask inputs
(diagonal: lower-tri for both cores; first other-half block per chunk: ones
for parity 0, zeros for parity 1). Per-core algorithm (all matmuls bf16 with
f32 PSUM accumulation):
  phase 1: K^T[h,s], V^T[h,s] per own 512-column block, Q^T[h,t] for all
           blocks; V^T transposed on PE to V[s,h] and augmented with a ones
           column (Vhat) so the AV matmul also produces the softmax
           denominator.
  phase 2: per own 128-key chunk c, S^T[s,t] = K^T_c.T @ Q^T over two query
           ranges (own half from col 128c, other half from col 2048+128c),
           exp on ScalarE (PSUM->SBUF, bf16), masks on the two leading
           128-blocks.
  phase 3: per query tile j, O[t, 0:H+1] = sum_c P^T_c.T @ Vhat_c in PSUM;
           staged 4 tiles per SBUF buffer and written with one batched DMA
           (no on-device normalize).

Schedule notes (all verified against the CoreSim cost model + hardware):
  - Q-only groups run in REVERSE order (7,6,5,4): block 7 has the largest
    score/exp/AV volume, so its ~10us of ScalarE exp overlaps the remaining
    Q projections; the final group (block 4) has only ~1.3k exp columns.
  - The final group's Q projection runs as two sequential 256-col sub-chains
    (PSUM accumulation groups are PER BANK, so parallel sub-chains in one
    bank are illegal); its score matmuls pack several chunks per PSUM bank
    as 1-matmul groups so one exp per bank covers them, staged via ptail.
  - Own-tile AV batches 0..3 / 4..7 are deferred into the late phase as PE
    filler where exp paces the machine.
  - One 128-col PE warmup matmul anchors the p-state ramp clock just before
    real work (the ramp resets if the PE idles, so warming up too early or
    too long regresses).
  - Weight/mask DMAs ride the Pool queue; the SP queue is the pure xt
    stream + batched output DMAs (DMA transfer time serializes per queue).
  - GPSIMD cannot touch PSUM on hardware (BIR verifier), so all PSUM->SBUF
    copies stay on DVE.
  - A pair-AllGather of the duplicated Q half was prototyped (28us model
    cost, overlappable in principle) but the Tile scheduler orders the
    long-latency dependent work ahead of ready work on the in-order engine
    streams, costing more than the 6.8us of saved projection; abandoned.
"""

import numpy as np
import ml_dtypes

B, T, D, H = 4, 4096, 2048, 128
N_CORES = 8
P = 128  # partitions

bf16 = ml_dtypes.bfloat16


def build_nc(d=D, tkv=T, h=H):
    import concourse.tile as tile
    from concourse import bacc, mybir

    assert h == P
    n_d = d // P          # 16 contraction chunks
    n_g = tkv // 512      # 8 column groups of xT
    n_gkv = n_g // 2      # 4 groups carrying own-half keys (K/V + Q)
    n_sc = tkv // 2 // P  # 16 own key chunks
    n_qt = tkv // P       # 32 local query tiles
    scale = 1.0 / float(np.sqrt(h))
    BF = mybir.dt.bfloat16
    F32 = mybir.dt.float32

    nc = bacc.Bacc("TRN2", target_bir_lowering=False, debug=False,
                   num_devices=N_CORES)

    xT_ext = nc.dram_tensor("xT", [d, tkv], BF, kind="ExternalInput").ap()
    wq_ext = nc.dram_tensor("wq_pre", [P, d], BF, kind="ExternalInput").ap()
    wk_ext = nc.dram_tensor("wk_pre", [P, d], BF, kind="ExternalInput").ap()
    wv_ext = nc.dram_tensor("wv_pre", [P, d], BF, kind="ExternalInput").ap()
    # masks and identity packed into one tensor -> one DMA
    mi_ext = nc.dram_tensor("mask_id", [P, 3 * P], BF, kind="ExternalInput").ap()
    # per-core partner-slot selectors: col0 = (parity==1), col1 = (parity==0)
    psel_ext = nc.dram_tensor("psel", [P, 2], F32, kind="ExternalInput").ap()
    out_ext = nc.dram_tensor("out", [tkv, h + 1], F32, kind="ExternalOutput").ap()

    with tile.TileContext(nc) as tc:
        with (
            tc.tile_pool(name="const", bufs=1) as const_pool,
            tc.tile_pool(name="persist", bufs=1) as persist,
            tc.tile_pool(name="xt", bufs=16) as xt_pool,
            tc.tile_pool(name="outp", bufs=6) as out_pool,
            tc.tile_pool(name="ps512", bufs=2, space="PSUM") as ps512,
            tc.tile_pool(name="pssm", bufs=2, space="PSUM") as pssm,
            tc.tile_pool(name="dram", bufs=1, space="DRAM") as dram_pool,
        ):
            # --- constants (only wk up front; wv/wq/masks stream in between
            # the first xt tiles so the PE can start earlier) ---
            w_sb = {}
            for name, ext in (("wq", wq_ext), ("wk", wk_ext), ("wv", wv_ext)):
                t_ = const_pool.tile([P, n_d * h], BF, tag=f"w_{name}", name=name)
                if name == "wq":
                    nc.gpsimd.dma_start(t_[:], ext[:])
                w_sb[name] = t_
            mi_sb = const_pool.tile([P, 3 * P], BF, tag="maskid")
            ps_sb = const_pool.tile([P, 2], F32, tag="psel")
            tri_sb = mi_sb[:, 0:P]          # lower-tri for diagonal blocks
            oth_sb = mi_sb[:, P:2 * P]      # ones (p=0) / zeros (p=1)
            id_sb = mi_sb[:, 2 * P:3 * P]

            def emit_late_consts(di):
                if di == 0:
                    nc.gpsimd.dma_start(w_sb["wk"][:], wk_ext[:])
                    nc.gpsimd.dma_start(w_sb["wv"][:], wv_ext[:])
                if di == 3:
                    nc.gpsimd.dma_start(mi_sb[:], mi_ext[:])
                    nc.gpsimd.dma_start(ps_sb[:], psel_ext[:])

            # --- PE warmup: throwaway matmuls during the DMA-bound head so
            # the p-state ramp is spent before real work ---
            warm = const_pool.tile([P, 512], BF, tag="warm")
            nc.gpsimd.memset(warm[:], 0.125)
            for _ in range(1):
                wu_ps = ps512.tile([P, 512], F32, tag="mm512", bufs=2,
                                   name="wu_ps")
                nc.tensor.matmul(wu_ps[:, 0:P], warm[:, 0:P], warm[:, 0:P],
                                 start=True, stop=True)

            # --- persistent activations ---
            kt_all = persist.tile([P, tkv // 2], BF, tag="kt")
            qt_all = persist.tile([P, tkv], BF, tag="qt")
            vhat = []
            for c in range(n_sc):
                vh = persist.tile([P, h + 1], BF, tag=f"vhat{c}", name=f"vh{c}")
                nc.gpsimd.memset(vh[:, h:h + 1], 1.0)
                vhat.append(vh)
            # pt[c]: exp(scores) for chunk c; own query range then other range
            pt = [persist.tile([P, 2 * (n_sc - c) * P], BF, tag=f"pt{c}",
                               name=f"pt{c}")
                  for c in range(n_sc)]
            # exp(scores) staging for the final group's four query tiles
            # (t16..t19), packed [c0t16 c0t17 c1t17 | c0t18 c0t19 c1t18 c1t19
            #  | c2t18 c2t19 c3t19]
            ptail = persist.tile([P, 10 * P], BF, tag="ptail")
            # staging for the AllGather result (both pair slots) and the
            # selected partner half; blocks 6,7 scores read qoth so the
            # collective path never writes qt_all (avoids WAR serialization
            # of unrelated qt_all readers behind the collective)
            qab = persist.tile([P, 2048], BF, tag="qab")
            qoth = persist.tile([P, 1024], BF, tag="qoth")

            qown_dram = dram_pool.tile([P, 1024], BF, tag="qown")
            qgath_dram = dram_pool.tile([2 * P, 1024], BF, tag="qgath")

            qt_blocks_ready = set()
            chunks_emitted = []
            scores_done = set()

            def _emit_score_piece(c, t0, w, pt_off, first, msk):
                if w <= 0:
                    return
                st_ps = ps512.tile([P, w], F32, tag="mm512", bufs=2,
                                   name="st_ps")
                # blocks 6,7 come from the pair exchange, staged in qoth
                mv = (qoth[:, t0 - 3072:t0 - 3072 + w] if t0 >= 3072
                      else qt_all[:, t0:t0 + w])
                nc.tensor.matmul(st_ps[:], kt_all[:, P * c:P * (c + 1)],
                                 mv, start=True, stop=True)
                nc.scalar.activation(pt[c][:, pt_off:pt_off + w], st_ps[:],
                                     mybir.ActivationFunctionType.Exp,
                                     scale=scale)
                if first:
                    base = pt_off
                    nc.vector.tensor_mul(pt[c][:, base:base + P],
                                         pt[c][:, base:base + P], msk)

            def _emit_score_block(c, b):
                # b: global 512-col qt block 0..7. Own range lives in blocks
                # [c//4, 4), other range in [4 + c//4, 8).
                if b < n_gkv:
                    q0 = P * c
                    t0 = max(q0, 512 * b)
                    pt_off = t0 - q0
                    msk = tri_sb
                else:
                    q0 = tkv // 2 + P * c
                    t0 = max(q0, 512 * b)
                    pt_off = (n_sc - c) * P + (t0 - q0)
                    msk = oth_sb
                w = 512 * (b + 1) - t0
                _emit_score_piece(c, t0, w, pt_off, t0 == q0, msk)

            def flush_scores():
                for c in chunks_emitted:
                    for b in sorted(qt_blocks_ready):
                        lo = c // 4 if b < n_gkv else n_gkv + c // 4
                        hi = n_gkv if b < n_gkv else n_g
                        if lo <= b < hi and (c, b) not in scores_done:
                            scores_done.add((c, b))
                            _emit_score_block(c, b)

            def emit_av(j, o_sb, slot):
                # local tile j: own tiles j<16 use chunks 0..j; other tiles
                # 16+i use chunks 0..i (the c==i block is data-masked).
                # Result staged into slot of a 4-tile buffer; the caller
                # issues one batched DMA per 4 tiles (amortizes the 500ns
                # descriptor-gen floor).
                i = j if j < n_sc else j - n_sc
                o_ps = pssm.tile([P, h + 1], F32, tag="small", name="o_ps")
                for c in range(i + 1):
                    off = (j - c) if j < n_sc else (n_sc - c) + (i - c)
                    nc.tensor.matmul(o_ps[:], pt[c][:, P * off:P * (off + 1)],
                                     vhat[c][:], start=(c == 0), stop=(c == i))
                nc.vector.tensor_copy(
                    o_sb[:, (h + 1) * slot:(h + 1) * (slot + 1)], o_ps[:])

            # --- phase 1: one pass over permuted xT; K/V for own half, Q
            # always; scores/AV interleaved as dependencies are emitted ---
            # Q-only groups run in REVERSE (7,6,5,4): the 512-col query block
            # with the largest score/exp/AV volume (b=7, all 16 chunks) is
            # produced first so its 10us of ScalarE exp overlaps the
            # remaining Q projections; the final group (b=4) has only ~1.3k
            # exp columns, shrinking the activation-serial tail.
            xT_r = xT_ext.rearrange("(a p) t -> p a t", p=P)
            n_pair = n_d // 2
            # Own groups 2,3 run FIRST with Q-first chains: their Q output
            # (own tiles 8..15) is the pair-exchange payload, so the
            # AllGather launches ~14us in and hides under the remaining
            # K/V/score work. Blocks 6,7 are never projected locally - they
            # arrive from the partner core.
            for g in [2, 3, 0, 1, 5, 4]:
                has_kv = g < n_gkv
                kt_ps = (ps512.tile([P, 512], F32, tag="acc", bufs=4, name="kt_ps")
                         if has_kv else None)
                vt_ps = (ps512.tile([P, 512], F32, tag="acc", bufs=4, name="vt_ps")
                         if has_kv else None)
                q_ps = ps512.tile([P, 512], F32, tag="acc", bufs=4, name="q_ps")

                def load_pair(pi, g=g):
                    # alternate the issuing queue: DMA transfer time is
                    # serialized per queue, and Pool is otherwise idle
                    xt = xt_pool.tile([P, 1024], BF, tag="xt", name="xt")
                    nc.sync.dma_start(
                        xt[:].rearrange("p (a t) -> p a t", t=512),
                        xT_r[:, 2 * pi:2 * pi + 2, 512 * g:512 * (g + 1)])
                    return xt

                def mm_pair(ps, wname, pi, xt):
                    for n in (0, 1):
                        di = 2 * pi + n
                        nc.tensor.matmul(ps[:],
                                         w_sb[wname][:, di * h:(di + 1) * h],
                                         xt[:, 512 * n:512 * (n + 1)],
                                         start=(di == 0), stop=(di == n_d - 1))

                if g == 2:
                    # first processed group: all Q matmuls first (wq is the
                    # one weight loaded up front); K/V chains follow
                    tiles = []
                    for pi in range(n_pair):
                        xt = load_pair(pi)
                        emit_late_consts(pi)
                        tiles.append(xt)
                        mm_pair(q_ps, "wq", pi, xt)
                    for pi, xt in enumerate(tiles):
                        mm_pair(kt_ps, "wk", pi, xt)
                    for pi, xt in enumerate(tiles):
                        mm_pair(vt_ps, "wv", pi, xt)
                elif g == 3:
                    # Q-first per pair: the exchange payload completes with
                    # the last xt pair
                    for pi in range(n_pair):
                        xt = load_pair(pi)
                        mm_pair(q_ps, "wq", pi, xt)
                        mm_pair(kt_ps, "wk", pi, xt)
                        mm_pair(vt_ps, "wv", pi, xt)
                elif g == n_gkv:
                    # FINAL group: the Q projection runs as two sequential
                    # 256-col sub-chains in one PSUM bank so the endgame
                    # scores/exp/AVs pipeline with the second sub-chain.
                    # Score matmuls for several chunks pack into shared PSUM
                    # banks (each a 1-matmul accumulation group) so ONE exp
                    # per bank covers them, writing the contiguous ptail
                    # staging that the last four AVs read.
                    tiles = [load_pair(pi) for pi in range(n_pair)]
                    nc.sync.dma_start(
                        qab[:].rearrange("p (a t) -> p a t", t=1024),
                        qgath_dram.opt().rearrange("(a p) t -> p a t", p=P))
                    for sub in (0, 1):
                        c0, c1 = 512 * g + 256 * sub, 512 * g + 256 * (sub + 1)
                        for pi, xt in enumerate(tiles):
                            for n in (0, 1):
                                di = 2 * pi + n
                                nc.tensor.matmul(
                                    q_ps[:, 256 * sub:256 * (sub + 1)],
                                    w_sb["wq"][:, di * h:(di + 1) * h],
                                    xt[:, 512 * n + 256 * sub:
                                       512 * n + 256 * (sub + 1)],
                                    start=(di == 0), stop=(di == n_d - 1))
                        nc.vector.tensor_copy(qt_all[:, c0:c1],
                                              q_ps[:, 256 * sub:256 * (sub + 1)])
                        if sub == 0:
                            # bank: [c0 cols 2048:2304 | c1 cols 2176:2304]
                            st = ps512.tile([P, 384], F32, tag="mm512", bufs=2,
                                            name="st_ps")
                            nc.tensor.matmul(st[:, 0:256],
                                             kt_all[:, 0:P],
                                             qt_all[:, c0:c1],
                                             start=True, stop=True)
                            nc.tensor.matmul(st[:, 256:384],
                                             kt_all[:, P:2 * P],
                                             qt_all[:, c0 + P:c1],
                                             start=True, stop=True)
                            nc.scalar.activation(
                                ptail[:, 0:384], st[:],
                                mybir.ActivationFunctionType.Exp, scale=scale)
                            nc.vector.tensor_mul(ptail[:, 0:P],
                                                 ptail[:, 0:P], oth_sb)
                            nc.vector.tensor_mul(ptail[:, 256:384],
                                                 ptail[:, 256:384], oth_sb)
                        else:
                            # bank B first ([c2 cols 2304:2560 | c3 cols
                            # 2432:2560]) so its mask multiplies overlap the
                            # bank A exp; bank A: [c0 | c1] x cols 2304:2560
                            stb = ps512.tile([P, 384], F32, tag="mm512",
                                             bufs=2, name="st_ps")
                            nc.tensor.matmul(stb[:, 0:256],
                                             kt_all[:, 2 * P:3 * P],
                                             qt_all[:, c0:c1],
                                             start=True, stop=True)
                            nc.tensor.matmul(stb[:, 256:384],
                                             kt_all[:, 3 * P:4 * P],
                                             qt_all[:, c0 + P:c1],
                                             start=True, stop=True)
                            nc.scalar.activation(
                                ptail[:, 896:1280], stb[:],
                                mybir.ActivationFunctionType.Exp, scale=scale)
                            nc.vector.tensor_mul(ptail[:, 896:896 + P],
                                                 ptail[:, 896:896 + P], oth_sb)
                            nc.vector.tensor_mul(ptail[:, 1152:1280],
                                                 ptail[:, 1152:1280], oth_sb)
                            sta = ps512.tile([P, 512], F32, tag="mm512",
                                             bufs=2, name="st_ps")
                            for ci in (0, 1):
                                nc.tensor.matmul(sta[:, 256 * ci:256 * (ci + 1)],
                                                 kt_all[:, P * ci:P * (ci + 1)],
                                                 qt_all[:, c0:c1],
                                                 start=True, stop=True)
                            nc.scalar.activation(
                                ptail[:, 384:896], sta[:],
                                mybir.ActivationFunctionType.Exp, scale=scale)
                else:
                    for pi in range(n_pair):
                        xt = load_pair(pi)
                        if has_kv:
                            mm_pair(kt_ps, "wk", pi, xt)
                            mm_pair(vt_ps, "wv", pi, xt)
                        mm_pair(q_ps, "wq", pi, xt)

                if g != n_gkv:
                    nc.vector.tensor_copy(qt_all[:, 512 * g:512 * (g + 1)],
                                          q_ps[:])
                    qt_blocks_ready.add(g)
                if has_kv:
                    nc.vector.tensor_copy(kt_all[:, 512 * g:512 * (g + 1)],
                                          kt_ps[:])
                if has_kv:
                    vt_sb = out_pool.tile([P, 512], BF, tag="vt", name="vt_sb")
                    nc.vector.tensor_copy(vt_sb[:], vt_ps[:])
                    for i in range(4):
                        c = 4 * g + i
                        vch_ps = ps512.tile([P, P], BF, tag="mm512", bufs=2,
                                            name="vch_ps")
                        nc.tensor.transpose(vch_ps[:],
                                            vt_sb[:, P * i:P * (i + 1)], id_sb)
                        nc.vector.tensor_copy(vhat[c][:, 0:h], vch_ps[:])
                        chunks_emitted.append(c)
                flush_scores()
                # AV for the tiles whose last dependency is this group:
                # own tiles 4g..4g+3 (g<4), other tiles 16+4(g-4).. (g>=4);
                # the final group runs in reverse so the very last AV is the
                # 1-chunk tile (shortest exp->AV->out chain)
                def _flush_pair(o_sb, base, b0):
                    nc.sync.dma_start(
                        out_ext[P * b0:P * (b0 + 2), :].rearrange(
                            "(a p) c -> p a c", p=P),
                        o_sb[:, (h + 1) * (b0 - base):
                             (h + 1) * (b0 - base + 2)].rearrange(
                            "p (a c) -> p a c", c=h + 1))

                def emit_av_batch(base):
                    o_sb = out_pool.tile([P, 4 * (h + 1)], F32, tag="osb",
                                         name="o_sb")
                    for j in range(base, base + 4):
                        emit_av(j, o_sb, j - base)
                    nc.sync.dma_start(
                        out_ext[P * base:P * (base + 4), :].rearrange(
                            "(a p) c -> p a c", p=P),
                        o_sb[:].rearrange("p (a c) -> p a c", c=h + 1))

                # Exchange: after group 3 the payload (own tiles 8..15)
                # goes out on the Pool queue; the receive DMA rides the
                # Activation queue and the partner-slot select stays on Pool
                # so neither pollutes a busy engine stream. All gather-gated
                # score/AV work is emitted DEAD LAST (after the final
                # group's body) - the per-engine streams are near-in-order
                # with only 4-deep dependency lookahead, so anything gated
                # on the collective must sit behind all independent work.
                if g == 3:
                    nc.gpsimd.dma_start(qown_dram[:], qt_all[:, 1024:2048])
                    nc.gpsimd.collective_compute(
                        "AllGather", mybir.AluOpType.bypass,
                        replica_groups=[[0, 1], [2, 3], [4, 5], [6, 7]],
                        ins=[qown_dram.opt()], outs=[qgath_dram.opt()])
                elif g == 0:
                    emit_av_batch(0)
                elif g == 1:
                    emit_av_batch(8)
                    emit_av_batch(12)
                elif g == 5:
                    emit_av_batch(n_sc + 4)
                    emit_av_batch(4)        # deferred own 4..7
                elif g == n_gkv:
                    def emit_av_tail(i, o_sb, slot, offs, eng=None):
                        o_ps = pssm.tile([P, h + 1], F32, tag="small",
                                         name="o_ps")
                        for ci, off in enumerate(offs):
                            nc.tensor.matmul(o_ps[:], ptail[:, off:off + P],
                                             vhat[ci][:], start=(ci == 0),
                                             stop=(ci == len(offs) - 1))
                        (eng or nc.vector).tensor_copy(
                            o_sb[:, (h + 1) * slot:(h + 1) * (slot + 1)],
                            o_ps[:])
                    o_sb = out_pool.tile([P, 4 * (h + 1)], F32, tag="osb",
                                         name="o_sb")
                    emit_av_tail(1, o_sb, 1, [P, 2 * P])
                    emit_av_tail(0, o_sb, 0, [0])
                    _flush_pair(o_sb, n_sc, n_sc)
                    emit_av_tail(3, o_sb, 3, [4 * P, 6 * P, 8 * P, 9 * P])
                    emit_av_tail(2, o_sb, 2, [3 * P, 5 * P, 7 * P])
                    _flush_pair(o_sb, n_sc, n_sc + 2)
                    # gather-gated blocks 6,7: scores, exps and AVs land
                    # here, after every piece of independent work
                    nc.gpsimd.tensor_scalar_mul(
                        qoth[:], qab[:, 0:1024], ps_sb[:, 0:1])
                    nc.gpsimd.scalar_tensor_tensor(
                        qoth[:], qab[:, 1024:2048],
                        ps_sb[:, 1:2], qoth[:],
                        mybir.AluOpType.mult, mybir.AluOpType.add)
                    # chunks ascending, AV batches as their chunk prefix
                    # completes, so AVs pipeline with the exps instead of
                    # trailing them all
                    for c in range(n_sc):
                        for b in (6, 7):
                            if n_gkv + c // 4 <= b:
                                _emit_score_block(c, b)
                        if c == 11:
                            emit_av_batch(n_sc + 8)
                    emit_av_batch(n_sc + 12)

    nc.compile()
    return nc


_NC_CACHE = {}


def _get_nc(d, tkv, h):
    key = (d, tkv, h)
    if key not in _NC_CACHE:
        _NC_CACHE[key] = build_nc(d, tkv, h)
    return _NC_CACHE[key]


def make_in_maps(x, Wq, Wk, Wv):
    """Shard full inputs into per-core input maps (host-side prep)."""
    x = np.asarray(x, dtype=np.float32)
    b_, t_, d_ = x.shape
    wq = np.asarray(Wq, dtype=np.float32).astype(bf16)
    wk = np.asarray(Wk, dtype=np.float32).astype(bf16)
    wv = np.asarray(Wv, dtype=np.float32).astype(bf16)

    def prearrange(w):
        # w_pre[p, n*h + j] = w[n*128 + p, j] -> matches the SBUF layout so the
        # weight DMA is a single contiguous transfer
        n_d = w.shape[0] // P
        return np.ascontiguousarray(
            w.reshape(n_d, P, w.shape[1]).transpose(1, 0, 2).reshape(P, -1))

    wq_pre, wk_pre, wv_pre = prearrange(wq), prearrange(wk), prearrange(wv)
    tri = (np.arange(P)[None, :] >= np.arange(P)[:, None])  # [s,t]: t>=s
    t0m = tri.astype(bf16)
    ones = np.ones((P, P), dtype=bf16)
    zeros = np.zeros((P, P), dtype=bf16)
    ident = np.eye(P, dtype=bf16)
    in_maps = []
    for core in range(2 * b_):
        b, p = core // 2, core % 2
        xb16 = x[b].astype(bf16)  # [T, D]
        # permute rows: own-parity 128-blocks first, then the others
        xbb = xb16.reshape(t_ // P, P, d_)
        xperm = np.concatenate([xbb[p::2], xbb[1 - p::2]], axis=0)
        xT_perm = np.ascontiguousarray(xperm.reshape(t_, d_).T)  # [D, T]
        mask_id = np.concatenate(
            [t0m, ones if p == 0 else zeros, ident], axis=1)
        psel = np.zeros((P, 2), dtype=np.float32)
        psel[:, 0] = 1.0 if p == 1 else 0.0   # partner is rank slot 0
        psel[:, 1] = 1.0 if p == 0 else 0.0   # partner is rank slot 1
        in_maps.append({
            "xT": xT_perm,
            "wq_pre": wq_pre, "wk_pre": wk_pre, "wv_pre": wv_pre,
            "mask_id": np.ascontiguousarray(mask_id),
            "psel": psel,
        })
    return in_maps


def gather_out(results, b_=B, t_=T, h_=H):
    """Unpermute per-core partials, combine the pair, normalize."""
    out = np.empty((b_, t_, h_), dtype=np.float32)
    n_blocks = t_ // P
    for b in range(b_):
        acc = np.zeros((n_blocks, P, h_ + 1), dtype=np.float32)
        for p in (0, 1):
            loc = results[2 * b + p]["out"].reshape(n_blocks, P, h_ + 1)
            # local tile j<16 -> global block 2j+p; 16+i -> global 2i+(1-p)
            acc[p::2] += loc[:n_blocks // 2]
            acc[1 - p::2] += loc[n_blocks // 2:]
        out[b] = (acc[:, :, :h_] / acc[:, :, h_:h_ + 1]).reshape(t_, h_)
    return out


def kernel(x, Wq, Wk, Wv):
    from concourse.bass_utils import run_bass_kernel_spmd

    nc = _get_nc(D, T, H)
    in_maps = make_in_maps(x, Wq, Wk, Wv)
    res = run_bass_kernel_spmd(nc, in_maps, core_ids=list(range(N_CORES)))
    return gather_out(res.results)


# revision 93
# speedup vs baseline: 1.0519x; 1.0519x over previous
"""Causal single-head attention on 8 TRN2 NeuronCores.

Problem: x[B=4,T=4096,D=2048] @ Wq/Wk/Wv[D,H=128] -> causal attention -> out[B,T,H].

Sharding (v6): 2 cores per batch, split by KEY parity with host-side softmax
combine. Core parity p owns the interleaved 128-row KEY blocks 2c+p
(c = 0..15): it computes K/V projections for its own key half only, Q for ALL
queries, scores+AV of every query against its own keys, and writes the
UNNORMALIZED partial numerator plus denominator [T, H+1]. The host sums the
two partials of each batch and divides. This duplicates the Q projection
(cheap, 1x) instead of K/V projection + transpose (2x+), cutting per-core PE
work ~17% vs the query-parity scheme (~203k PE cycles/core vs ~246k).

The host permutes each batch's rows to [own-parity 128-blocks | other
blocks], transposes and casts to bf16, so one 16MB xT stream feeds all three
projections (K/V over the first half only). The permuted causal structure is
core-independent; per-core causality lives in two 128x128 mask inputs
(diagonal: lower-tri for both cores; first other-half block per chunk: ones
for parity 0, zeros for parity 1). Per-core algorithm (all matmuls bf16 with
f32 PSUM accumulation):
  phase 1: K^T[h,s], V^T[h,s] per own 512-column block, Q^T[h,t] for all
           blocks; V^T transposed on PE to V[s,h] and augmented with a ones
           column (Vhat) so the AV matmul also produces the softmax
           denominator.
  phase 2: per own 128-key chunk c, S^T[s,t] = K^T_c.T @ Q^T over two query
           ranges (own half from col 128c, other half from col 2048+128c),
           exp on ScalarE (PSUM->SBUF, bf16), masks on the two leading
           128-blocks.
  phase 3: per query tile j, O[t, 0:H+1] = sum_c P^T_c.T @ Vhat_c in PSUM;
           staged 4 tiles per SBUF buffer and written with one batched DMA
           (no on-device normalize).

Schedule notes (all verified against the CoreSim cost model + hardware):
  - Q-only groups run in REVERSE order (7,6,5,4): block 7 has the largest
    score/exp/AV volume, so its ~10us of ScalarE exp overlaps the remaining
    Q projections; the final group (block 4) has only ~1.3k exp columns.
  - The final group's Q projection runs as two sequential 256-col sub-chains
    (PSUM accumulation groups are PER BANK, so parallel sub-chains in one
    bank are illegal); its score matmuls pack several chunks per PSUM bank
    as 1-matmul groups so one exp per bank covers them, staged via ptail.
  - Own-tile AV batches 0..3 / 4..7 are deferred into the late phase as PE
    filler where exp paces the machine.
  - One 128-col PE warmup matmul anchors the p-state ramp clock just before
    real work (the ramp resets if the PE idles, so warming up too early or
    too long regresses).
  - Weight/mask DMAs ride the Pool queue; the SP queue is the pure xt
    stream + batched output DMAs (DMA transfer time serializes per queue).
  - GPSIMD cannot touch PSUM on hardware (BIR verifier), so all PSUM->SBUF
    copies stay on DVE.
  - A pair-AllGather of the duplicated Q half was prototyped (28us model
    cost, overlappable in principle) but the Tile scheduler orders the
    long-latency dependent work ahead of ready work on the in-order engine
    streams, costing more than the 6.8us of saved projection; abandoned.
"""

import numpy as np
import ml_dtypes

B, T, D, H = 4, 4096, 2048, 128
N_CORES = 8
P = 128  # partitions

bf16 = ml_dtypes.bfloat16


def build_nc(d=D, tkv=T, h=H):
    import concourse.tile as tile
    from concourse import bacc, mybir

    assert h == P
    n_d = d // P          # 16 contraction chunks
    n_g = tkv // 512      # 8 column groups of xT
    n_gkv = n_g // 2      # 4 groups carrying own-half keys (K/V + Q)
    n_sc = tkv // 2 // P  # 16 own key chunks
    n_qt = tkv // P       # 32 local query tiles
    scale = 1.0 / float(np.sqrt(h))
    BF = mybir.dt.bfloat16
    F32 = mybir.dt.float32

    nc = bacc.Bacc("TRN2", target_bir_lowering=False, debug=False,
                   num_devices=N_CORES)

    xT_ext = nc.dram_tensor("xT", [d, tkv], BF, kind="ExternalInput").ap()
    wq_ext = nc.dram_tensor("wq_pre", [P, d], BF, kind="ExternalInput").ap()
    wk_ext = nc.dram_tensor("wk_pre", [P, d], BF, kind="ExternalInput").ap()
    wv_ext = nc.dram_tensor("wv_pre", [P, d], BF, kind="ExternalInput").ap()
    # masks and identity packed into one tensor -> one DMA
    mi_ext = nc.dram_tensor("mask_id", [P, 3 * P], BF, kind="ExternalInput").ap()
    out_ext = nc.dram_tensor("out", [tkv, h + 1], F32, kind="ExternalOutput").ap()

    with tile.TileContext(nc) as tc:
        with (
            tc.tile_pool(name="const", bufs=1) as const_pool,
            tc.tile_pool(name="persist", bufs=1) as persist,
            tc.tile_pool(name="xt", bufs=16) as xt_pool,
            tc.tile_pool(name="outp", bufs=6) as out_pool,
            tc.tile_pool(name="ps512", bufs=2, space="PSUM") as ps512,
            tc.tile_pool(name="pssm", bufs=2, space="PSUM") as pssm,
        ):
            # --- constants (only wk up front; wv/wq/masks stream in between
            # the first xt tiles so the PE can start earlier) ---
            w_sb = {}
            for name, ext in (("wk", wk_ext), ("wv", wv_ext), ("wq", wq_ext)):
                t_ = const_pool.tile([P, n_d * h], BF, tag=f"w_{name}", name=name)
                if name == "wk":
                    nc.gpsimd.dma_start(t_[:], ext[:])
                w_sb[name] = t_
            mi_sb = const_pool.tile([P, 3 * P], BF, tag="maskid")
            tri_sb = mi_sb[:, 0:P]          # lower-tri for diagonal blocks
            oth_sb = mi_sb[:, P:2 * P]      # ones (p=0) / zeros (p=1)
            id_sb = mi_sb[:, 2 * P:3 * P]

            def emit_late_consts(di):
                if di == 0:
                    nc.gpsimd.dma_start(w_sb["wv"][:], wv_ext[:])
                    nc.gpsimd.dma_start(w_sb["wq"][:], wq_ext[:])
                if di == 3:
                    nc.gpsimd.dma_start(mi_sb[:], mi_ext[:])

            # --- PE warmup: throwaway matmuls during the DMA-bound head so
            # the p-state ramp is spent before real work ---
            warm = const_pool.tile([P, 512], BF, tag="warm")
            nc.gpsimd.memset(warm[:], 0.125)
            for _ in range(1):
                wu_ps = ps512.tile([P, 512], F32, tag="mm512", bufs=3,
                                   name="wu_ps")
                nc.tensor.matmul(wu_ps[:, 0:P], warm[:, 0:P], warm[:, 0:P],
                                 start=True, stop=True)

            # --- persistent activations ---
            kt_all = persist.tile([P, tkv // 2], BF, tag="kt")
            qt_all = persist.tile([P, tkv], BF, tag="qt")
            vhat = []
            for c in range(n_sc):
                vh = persist.tile([P, h + 1], BF, tag=f"vhat{c}", name=f"vh{c}")
                nc.gpsimd.memset(vh[:, h:h + 1], 1.0)
                vhat.append(vh)
            # pt[c]: exp(scores) for chunk c; own query range then other range
            pt = [persist.tile([P, 2 * (n_sc - c) * P], BF, tag=f"pt{c}",
                               name=f"pt{c}")
                  for c in range(n_sc)]
            # exp(scores) staging for the final group's four query tiles
            # (t16..t19), packed [c0t16 c0t17 c1t17 | c0t18 c0t19 c1t18 c1t19
            #  | c2t18 c2t19 c3t19]
            ptail = persist.tile([P, 10 * P], BF, tag="ptail")

            qt_blocks_ready = set()
            chunks_emitted = []
            scores_done = set()

            def _emit_score_piece(c, t0, w, pt_off, first, msk):
                if w <= 0:
                    return
                st_ps = ps512.tile([P, w], F32, tag="mm512", bufs=3,
                                   name="st_ps")
                nc.tensor.matmul(st_ps[:], kt_all[:, P * c:P * (c + 1)],
                                 qt_all[:, t0:t0 + w], start=True, stop=True)
                nc.scalar.activation(pt[c][:, pt_off:pt_off + w], st_ps[:],
                                     mybir.ActivationFunctionType.Exp,
                                     scale=scale)
                if first:
                    base = pt_off
                    nc.vector.tensor_mul(pt[c][:, base:base + P],
                                         pt[c][:, base:base + P], msk)

            def _emit_score_block(c, b):
                # b: global 512-col qt block 0..7. Own range lives in blocks
                # [c//4, 4), other range in [4 + c//4, 8).
                if b < n_gkv:
                    q0 = P * c
                    t0 = max(q0, 512 * b)
                    pt_off = t0 - q0
                    msk = tri_sb
                else:
                    q0 = tkv // 2 + P * c
                    t0 = max(q0, 512 * b)
                    pt_off = (n_sc - c) * P + (t0 - q0)
                    msk = oth_sb
                w = 512 * (b + 1) - t0
                _emit_score_piece(c, t0, w, pt_off, t0 == q0, msk)

            def flush_scores():
                for c in chunks_emitted:
                    for b in sorted(qt_blocks_ready):
                        lo = c // 4 if b < n_gkv else n_gkv + c // 4
                        hi = n_gkv if b < n_gkv else n_g
                        if lo <= b < hi and (c, b) not in scores_done:
                            scores_done.add((c, b))
                            _emit_score_block(c, b)

            def emit_av(j, o_sb, slot):
                # local tile j: own tiles j<16 use chunks 0..j; other tiles
                # 16+i use chunks 0..i (the c==i block is data-masked).
                # Result staged into slot of a 4-tile buffer; the caller
                # issues one batched DMA per 4 tiles (amortizes the 500ns
                # descriptor-gen floor).
                i = j if j < n_sc else j - n_sc
                o_ps = pssm.tile([P, h + 1], F32, tag="small", name="o_ps")
                for c in range(i + 1):
                    off = (j - c) if j < n_sc else (n_sc - c) + (i - c)
                    nc.tensor.matmul(o_ps[:], pt[c][:, P * off:P * (off + 1)],
                                     vhat[c][:], start=(c == 0), stop=(c == i))
                nc.vector.tensor_copy(
                    o_sb[:, (h + 1) * slot:(h + 1) * (slot + 1)], o_ps[:])

            # --- phase 1: one pass over permuted xT; K/V for own half, Q
            # always; scores/AV interleaved as dependencies are emitted ---
            # Q-only groups run in REVERSE (7,6,5,4): the 512-col query block
            # with the largest score/exp/AV volume (b=7, all 16 chunks) is
            # produced first so its 10us of ScalarE exp overlaps the
            # remaining Q projections; the final group (b=4) has only ~1.3k
            # exp columns, shrinking the activation-serial tail.
            xT_r = xT_ext.rearrange("(a p) t -> p a t", p=P)
            n_pair = n_d // 2
            for g in [0, 1, 2, 3, 7, 6, 5, 4]:
                has_kv = g < n_gkv
                kt_ps = (ps512.tile([P, 512], F32, tag="acc", bufs=3, name="kt_ps")
                         if has_kv else None)
                vt_ps = (ps512.tile([P, 512], F32, tag="acc", bufs=3, name="vt_ps")
                         if has_kv else None)
                q_ps = ps512.tile([P, 512], F32, tag="acc", bufs=3, name="q_ps")

                def load_pair(pi, g=g):
                    # alternate the issuing queue: DMA transfer time is
                    # serialized per queue, and Pool is otherwise idle
                    xt = xt_pool.tile([P, 1024], BF, tag="xt", name="xt")
                    nc.sync.dma_start(
                        xt[:].rearrange("p (a t) -> p a t", t=512),
                        xT_r[:, 2 * pi:2 * pi + 2, 512 * g:512 * (g + 1)])
                    return xt

                def mm_pair(ps, wname, pi, xt):
                    for n in (0, 1):
                        di = 2 * pi + n
                        nc.tensor.matmul(ps[:],
                                         w_sb[wname][:, di * h:(di + 1) * h],
                                         xt[:, 512 * n:512 * (n + 1)],
                                         start=(di == 0), stop=(di == n_d - 1))

                if g == 0:
                    # all K matmuls first (they only need wk, the one weight
                    # loaded up front); V/Q groups follow. The first pair is
                    # DMA'd as two 512-col singles so the very first K matmul
                    # starts one DMA-latency earlier
                    tiles = []
                    for pi in range(n_pair):
                        if pi == 0:
                            xt = xt_pool.tile([P, 1024], BF, tag="xt",
                                              name="xt")
                            for n in (0, 1):
                                nc.sync.dma_start(
                                    xt[:, 512 * n:512 * (n + 1)],
                                    xT_r[:, 2 * pi + n, 0:512])
                        else:
                            xt = load_pair(pi)
                        emit_late_consts(pi)
                        tiles.append(xt)
                        mm_pair(kt_ps, "wk", pi, xt)
                    for pi, xt in enumerate(tiles):
                        mm_pair(vt_ps, "wv", pi, xt)
                    for pi, xt in enumerate(tiles):
                        mm_pair(q_ps, "wq", pi, xt)
                elif g == n_gkv:
                    # FINAL group: the Q projection runs as two sequential
                    # 256-col sub-chains in one PSUM bank so the endgame
                    # scores/exp/AVs pipeline with the second sub-chain.
                    # Score matmuls for several chunks pack into shared PSUM
                    # banks (each a 1-matmul accumulation group) so ONE exp
                    # per bank covers them, writing the contiguous ptail
                    # staging that the last four AVs read.
                    tiles = [load_pair(pi) for pi in range(n_pair)]
                    for sub in (0, 1):
                        c0, c1 = 512 * g + 256 * sub, 512 * g + 256 * (sub + 1)
                        for pi, xt in enumerate(tiles):
                            for n in (0, 1):
                                di = 2 * pi + n
                                nc.tensor.matmul(
                                    q_ps[:, 256 * sub:256 * (sub + 1)],
                                    w_sb["wq"][:, di * h:(di + 1) * h],
                                    xt[:, 512 * n + 256 * sub:
                                       512 * n + 256 * (sub + 1)],
                                    start=(di == 0), stop=(di == n_d - 1))
                        nc.vector.tensor_copy(qt_all[:, c0:c1],
                                              q_ps[:, 256 * sub:256 * (sub + 1)])
                        if sub == 0:
                            # bank: [c0 cols 2048:2304 | c1 cols 2176:2304]
                            st = ps512.tile([P, 384], F32, tag="mm512", bufs=3,
                                            name="st_ps")
                            nc.tensor.matmul(st[:, 0:256],
                                             kt_all[:, 0:P],
                                             qt_all[:, c0:c1],
                                             start=True, stop=True)
                            nc.tensor.matmul(st[:, 256:384],
                                             kt_all[:, P:2 * P],
                                             qt_all[:, c0 + P:c1],
                                             start=True, stop=True)
                            nc.scalar.activation(
                                ptail[:, 0:384], st[:],
                                mybir.ActivationFunctionType.Exp, scale=scale)
                            nc.vector.tensor_mul(ptail[:, 0:P],
                                                 ptail[:, 0:P], oth_sb)
                            nc.vector.tensor_mul(ptail[:, 256:384],
                                                 ptail[:, 256:384], oth_sb)
                        else:
                            # bank B first ([c2 cols 2304:2560 | c3 cols
                            # 2432:2560]) so its mask multiplies overlap the
                            # bank A exp; bank A: [c0 | c1] x cols 2304:2560
                            stb = ps512.tile([P, 384], F32, tag="mm512",
                                             bufs=3, name="st_ps")
                            nc.tensor.matmul(stb[:, 0:256],
                                             kt_all[:, 2 * P:3 * P],
                                             qt_all[:, c0:c1],
                                             start=True, stop=True)
                            nc.tensor.matmul(stb[:, 256:384],
                                             kt_all[:, 3 * P:4 * P],
                                             qt_all[:, c0 + P:c1],
                                             start=True, stop=True)
                            nc.scalar.activation(
                                ptail[:, 896:1280], stb[:],
                                mybir.ActivationFunctionType.Exp, scale=scale)
                            nc.vector.tensor_mul(ptail[:, 896:896 + P],
                                                 ptail[:, 896:896 + P], oth_sb)
                            nc.vector.tensor_mul(ptail[:, 1152:1280],
                                                 ptail[:, 1152:1280], oth_sb)
                            sta = ps512.tile([P, 512], F32, tag="mm512",
                                             bufs=3, name="st_ps")
                            for ci in (0, 1):
                                nc.tensor.matmul(sta[:, 256 * ci:256 * (ci + 1)],
                                                 kt_all[:, P * ci:P * (ci + 1)],
                                                 qt_all[:, c0:c1],
                                                 start=True, stop=True)
                            nc.scalar.activation(
                                ptail[:, 384:896], sta[:],
                                mybir.ActivationFunctionType.Exp, scale=scale)
                else:
                    for pi in range(n_pair):
                        xt = load_pair(pi)
                        if has_kv:
                            mm_pair(kt_ps, "wk", pi, xt)
                            mm_pair(vt_ps, "wv", pi, xt)
                        mm_pair(q_ps, "wq", pi, xt)

                if has_kv:
                    nc.vector.tensor_copy(kt_all[:, 512 * g:512 * (g + 1)],
                                          kt_ps[:])
                if g != n_gkv:
                    nc.vector.tensor_copy(qt_all[:, 512 * g:512 * (g + 1)],
                                          q_ps[:])
                    qt_blocks_ready.add(g)
                if has_kv:
                    vt_sb = out_pool.tile([P, 512], BF, tag="vt", name="vt_sb")
                    nc.vector.tensor_copy(vt_sb[:], vt_ps[:])
                    for i in range(4):
                        c = 4 * g + i
                        vch_ps = ps512.tile([P, P], BF, tag="mm512", bufs=3,
                                            name="vch_ps")
                        nc.tensor.transpose(vch_ps[:],
                                            vt_sb[:, P * i:P * (i + 1)], id_sb)
                        nc.vector.tensor_copy(vhat[c][:, 0:h], vch_ps[:])
                        chunks_emitted.append(c)
                flush_scores()
                # AV for the tiles whose last dependency is this group:
                # own tiles 4g..4g+3 (g<4), other tiles 16+4(g-4).. (g>=4);
                # the final group runs in reverse so the very last AV is the
                # 1-chunk tile (shortest exp->AV->out chain)
                def _flush_pair(o_sb, base, b0):
                    nc.sync.dma_start(
                        out_ext[P * b0:P * (b0 + 2), :].rearrange(
                            "(a p) c -> p a c", p=P),
                        o_sb[:, (h + 1) * (b0 - base):
                             (h + 1) * (b0 - base + 2)].rearrange(
                            "p (a c) -> p a c", c=h + 1))

                def emit_av_batch(base):
                    o_sb = out_pool.tile([P, 4 * (h + 1)], F32, tag="osb",
                                         name="o_sb")
                    for j in range(base, base + 4):
                        emit_av(j, o_sb, j - base)
                    nc.sync.dma_start(
                        out_ext[P * base:P * (base + 4), :].rearrange(
                            "(a p) c -> p a c", p=P),
                        o_sb[:].rearrange("p (a c) -> p a c", c=h + 1))

                # Own tiles 0..7 (tiny AV chains, exp long done) are DEFERRED
                # into the activation-paced endgame as PE filler; the final
                # group's AVs pair with its two sub-chains, shortest chain
                # last
                if g in (2, 3):
                    emit_av_batch(4 * g)
                elif g == 7:
                    emit_av_batch(n_sc + 12)
                elif g == 6:
                    emit_av_batch(n_sc + 8)
                    emit_av_batch(4)        # deferred own 4..7
                elif g == 5:
                    emit_av_batch(n_sc + 4)
                    emit_av_batch(0)        # deferred own 0..3
                elif g == n_gkv:
                    def emit_av_tail(i, o_sb, slot, offs, eng=None):
                        o_ps = pssm.tile([P, h + 1], F32, tag="small",
                                         name="o_ps")
                        for ci, off in enumerate(offs):
                            nc.tensor.matmul(o_ps[:], ptail[:, off:off + P],
                                             vhat[ci][:], start=(ci == 0),
                                             stop=(ci == len(offs) - 1))
                        (eng or nc.vector).tensor_copy(
                            o_sb[:, (h + 1) * slot:(h + 1) * (slot + 1)],
                            o_ps[:])
                    o_sb = out_pool.tile([P, 4 * (h + 1)], F32, tag="osb",
                                         name="o_sb")
                    emit_av_tail(1, o_sb, 1, [P, 2 * P])
                    emit_av_tail(0, o_sb, 0, [0])
                    _flush_pair(o_sb, n_sc, n_sc)
                    emit_av_tail(3, o_sb, 3, [4 * P, 6 * P, 8 * P, 9 * P])
                    emit_av_tail(2, o_sb, 2, [3 * P, 5 * P, 7 * P])
                    _flush_pair(o_sb, n_sc, n_sc + 2)

    nc.compile()
    return nc


_NC_CACHE = {}


def _get_nc(d, tkv, h):
    key = (d, tkv, h)
    if key not in _NC_CACHE:
        _NC_CACHE[key] = build_nc(d, tkv, h)
    return _NC_CACHE[key]


def make_in_maps(x, Wq, Wk, Wv):
    """Shard full inputs into per-core input maps (host-side prep)."""
    x = np.asarray(x, dtype=np.float32)
    b_, t_, d_ = x.shape
    wq = np.asarray(Wq, dtype=np.float32).astype(bf16)
    wk = np.asarray(Wk, dtype=np.float32).astype(bf16)
    wv = np.asarray(Wv, dtype=np.float32).astype(bf16)

    def prearrange(w):
        # w_pre[p, n*h + j] = w[n*128 + p, j] -> matches the SBUF layout so the
        # weight DMA is a single contiguous transfer
        n_d = w.shape[0] // P
        return np.ascontiguousarray(
            w.reshape(n_d, P, w.shape[1]).transpose(1, 0, 2).reshape(P, -1))

    wq_pre, wk_pre, wv_pre = prearrange(wq), prearrange(wk), prearrange(wv)
    tri = (np.arange(P)[None, :] >= np.arange(P)[:, None])  # [s,t]: t>=s
    t0m = tri.astype(bf16)
    ones = np.ones((P, P), dtype=bf16)
    zeros = np.zeros((P, P), dtype=bf16)
    ident = np.eye(P, dtype=bf16)
    in_maps = []
    for core in range(2 * b_):
        b, p = core // 2, core % 2
        xb16 = x[b].astype(bf16)  # [T, D]
        # permute rows: own-parity 128-blocks first, then the others
        xbb = xb16.reshape(t_ // P, P, d_)
        xperm = np.concatenate([xbb[p::2], xbb[1 - p::2]], axis=0)
        xT_perm = np.ascontiguousarray(xperm.reshape(t_, d_).T)  # [D, T]
        mask_id = np.concatenate(
            [t0m, ones if p == 0 else zeros, ident], axis=1)
        in_maps.append({
            "xT": xT_perm,
            "wq_pre": wq_pre, "wk_pre": wk_pre, "wv_pre": wv_pre,
            "mask_id": np.ascontiguousarray(mask_id),
        })
    return in_maps


def gather_out(results, b_=B, t_=T, h_=H):
    """Unpermute per-core partials, combine the pair, normalize."""
    out = np.empty((b_, t_, h_), dtype=np.float32)
    n_blocks = t_ // P
    for b in range(b_):
        acc = np.zeros((n_blocks, P, h_ + 1), dtype=np.float32)
        for p in (0, 1):
            loc = results[2 * b + p]["out"].reshape(n_blocks, P, h_ + 1)
            # local tile j<16 -> global block 2j+p; 16+i -> global 2i+(1-p)
            acc[p::2] += loc[:n_blocks // 2]
            acc[1 - p::2] += loc[n_blocks // 2:]
        out[b] = (acc[:, :, :h_] / acc[:, :, h_:h_ + 1]).reshape(t_, h_)
    return out


def kernel(x, Wq, Wk, Wv):
    from concourse.bass_utils import run_bass_kernel_spmd

    nc = _get_nc(D, T, H)
    in_maps = make_in_maps(x, Wq, Wk, Wv)
    res = run_bass_kernel_spmd(nc, in_maps, core_ids=list(range(N_CORES)))
    return gather_out(res.results)
